# revision 35
# baseline (speedup 1.0000x reference)
"""Tensor-parallel (head-sharded) Llama-style attention layer for 8 NeuronCores.

Problem shapes (hardcoded): B=2, S=2048, D=4096, NH=32 q-heads, NKV=8 kv-heads,
HD=128, causal prefill (input_pos == arange(S), mask == tril).

Sharding: core i gets q-heads 4i..4i+3 and kv-head i (wq/wk/wv output dims and
wo input dims sharded by head). x is replicated. Each core produces a partial
final output (its heads' contribution through wo); the host sums the 8 partials
(the "all-reduce after wo" done on host since the kernel returns full output).

v3 layout/precision strategy (from NTFF trace analysis):
  - all matmul operands in bf16: LDWEIGHTS halves (224->~112ns) and hides
    under the 213ns matmul stream; DMA traffic halves. PSUM stays f32.
  - softmax denominator accumulated on DVE (e_sum += exp tile, bf16 2x/4x
    mode) instead of per-tile ones-matmuls on the PE; one full-ones-stationary
    matmul per (b,qb,h) both reduces over partitions AND broadcasts the
    rowsum to 128 partitions; reciprocal_approx_fast (5x) normalizes.
  - output-projection (u,ob) quads are interleaved into the attention kt
    loop between the scores and PV matmuls, filling the PE gap while the
    ScalarE exp (measured ~824ns/tile) produces the PV input.
  - q-block / kv loads issue on the Activation DGE so they never sit behind
    the 8MB/block output stores on the SP DGE queue.
"""

import math
from collections import deque
from contextlib import ExitStack

import numpy as np

B, S, D = 2, 2048, 4096
NH, NKV, HD = 32, 8, 128
NCORES = 8
QH = NH // NCORES  # q heads per core
EQ = QH * HD  # 512 = per-core q/o head-dim width
T = B * S  # 4096 total tokens
TB = 512  # token block (phase 1 / q blocks)
NTB = T // TB  # 8
DCH = D // 128  # 32 contraction chunks over model dim
NKT = S // 128  # 16 k tiles per batch
SCALE = 1.0 / math.sqrt(HD)

_NC_CACHE = {}


def _emit_phase1(nc, tc, ph1, mybir, tens, scratch):
    """QKV projections + RoPE + v transpose (all matmul operands bf16)."""
    F32 = mybir.dt.float32
    BF16 = mybir.dt.bfloat16
    xT, wqkvT = tens["xT"], tens["wqkvT"]
    cosd, sind = tens["cosT"], tens["sinT"]
    identr_t = tens["identr_t"]
    kTs, vs = scratch["kTs"], scratch["vs"]
    woT = tens["woT"]
    WQKV = EQ + 2 * HD  # 768 fused q|k|v output dims per core

    wpool = ph1.enter_context(tc.tile_pool(name="w1", bufs=1))
    w_c = [
        wpool.tile([128, WQKV], BF16, tag=f"wc{c}", name=f"w_c{c}")
        for c in range(DCH)
    ]

    xp = ph1.enter_context(tc.tile_pool(name="xp", bufs=16))
    rp = ph1.enter_context(tc.tile_pool(name="rope", bufs=3))
    sp1 = ph1.enter_context(tc.tile_pool(name="sp1", bufs=3))
    pp1 = ph1.enter_context(tc.tile_pool(name="pp1", bufs=1, space="PSUM"))
    pt1 = ph1.enter_context(tc.tile_pool(name="pt1", bufs=2, space="PSUM"))

    for tb in range(NTB):
        t0 = tb * TB
        b = t0 // S
        ts0 = t0 % S

        psq = [
            pp1.tile([128, TB], F32, tag=f"q{j}", name=f"psq{j}")
            for j in range(QH)
        ]
        psk = pp1.tile([128, TB], F32, tag="k")
        psv = pp1.tile([128, TB], F32, tag="v")
        for c in range(DCH):
            xc = xp.tile([128, TB], BF16, tag="x")
            if tb == 1:
                # tb=1's x also rides the Activation DGE: the SP queue is
                # still draining tb=0's weight burst at this point
                nc.scalar.dma_start(xc, xT[c * 128 : (c + 1) * 128, t0 : t0 + TB])
            elif tb == 0:
                # tb=0 is DMA-crunched (weights + x): x chunks go out on the
                # otherwise-idle Activation DGE so the PE never goes cold,
                # and the fused qkv weight tile is a single descriptor
                nc.scalar.dma_start(xc, xT[c * 128 : (c + 1) * 128, t0 : t0 + TB])
                nc.sync.dma_start(w_c[c], wqkvT[c * 128 : (c + 1) * 128, :])
            else:
                nc.sync.dma_start(xc, xT[c * 128 : (c + 1) * 128, t0 : t0 + TB])
            st = c == 0
            sp = c == DCH - 1
            for j in range(QH):
                nc.tensor.matmul(
                    psq[j],
                    w_c[c][:, j * 128 : (j + 1) * 128],
                    xc,
                    start=st,
                    stop=sp,
                )
            nc.tensor.matmul(psk, w_c[c][:, EQ : EQ + HD], xc, start=st, stop=sp)
            nc.tensor.matmul(
                psv, w_c[c][:, EQ + HD : EQ + 2 * HD], xc, start=st, stop=sp
            )

        # epilogue: rope q heads + k, transpose v
        cos_blk = rp.tile([128, TB], F32, tag="cos")
        sin_blk = rp.tile([128, TB], F32, tag="sin")
        nc.sync.dma_start(cos_blk, cosd[:, ts0 : ts0 + TB])
        nc.sync.dma_start(sin_blk, sind[:, ts0 : ts0 + TB])
        ectr = [0]

        def rope_emit(psum_in, tag, out_ap=None):
            src = sp1.tile([128, TB], F32, tag=f"src{tag}", name="src")
            if ectr[0] % 2 == 0:
                nc.scalar.copy(src, psum_in)
            else:
                nc.vector.tensor_copy(src, psum_in)
            ectr[0] += 1
            rot = rp.tile([128, TB], F32, tag="rot", name="rot")
            nc.vector.tensor_copy(rot[0:64, :], src[64:128, :])
            nc.vector.tensor_copy(rot[64:128, :], src[0:64, :])
            t1 = rp.tile([128, TB], F32, tag="t1", name="t1")
            nc.vector.tensor_mul(t1, src, cos_blk)
            nc.vector.tensor_mul(rot, rot, sin_blk)
            if out_ap is None:
                out_ap = rp.tile([128, TB], BF16, tag="ro", name="ro")
            nc.vector.tensor_add(out_ap, t1, rot)
            return out_ap

        qt_sb = scratch["shared"]["qt"][b]
        for j in range(QH):
            # roped q goes straight into the SBUF-resident q tensor — no
            # DRAM roundtrip, and phase 2 reads slices with no DMA at all
            rope_emit(psq[j], "q", qt_sb[:, j, ts0 : ts0 + TB])
        kr = rope_emit(psk, "k")
        nc.sync.dma_start(kTs[b][:, ts0 : ts0 + TB], kr)

        sv = sp1.tile([128, TB], BF16, tag="sv")
        nc.scalar.copy(sv, psv)
        for u in range(TB // 128):
            ptr = pt1.tile([128, 128], BF16, tag="vtr", name="ptr")
            nc.tensor.transpose(ptr, sv[:, u * 128 : (u + 1) * 128], identr_t)
            svT = sp1.tile([128, 128], BF16, tag="svT", name="svT")
            nc.scalar.copy(svT, ptr)
            nc.sync.dma_start(vs[b][ts0 + u * 128 : ts0 + (u + 1) * 128, :], svT)

        # prefetch phase-2 working set on the Activation DGE while the SP
        # queue streams x: kv cache of batch b right after its last store,
        # wo after tb 5 — so phase 2 starts with everything resident
        shared = scratch["shared"]
        if tb == 3:
            nc.scalar.dma_start(shared["kt"][0], kTs[0])
            nc.scalar.dma_start(
                shared["vt"][0], vs[0].rearrange("(n p) d -> p n d", p=128)
            )
        if tb == 5:
            for c in range(QH):
                nc.scalar.dma_start(
                    shared["wo_c"][c], woT[c * 128 : (c + 1) * 128, :]
                )


def _emit_phase23(nc, tc, ph2, mybir, tens, scratch):
    """Software-pipelined attention with interleaved output projection."""
    F32 = mybir.dt.float32
    BF16 = mybir.dt.bfloat16
    Exp = mybir.ActivationFunctionType.Exp
    ones_t = tens["ones_t"]
    kTs, vs = scratch["kTs"], scratch["vs"]
    out = tens["out"]

    ep = ph2.enter_context(tc.tile_pool(name="ep", bufs=8))
    esp = ph2.enter_context(tc.tile_pool(name="esp", bufs=2))
    sp2 = ph2.enter_context(tc.tile_pool(name="sp2", bufs=2))
    yp = ph2.enter_context(tc.tile_pool(name="yp", bufs=3))
    op = ph2.enter_context(tc.tile_pool(name="op", bufs=6))
    pps = ph2.enter_context(tc.tile_pool(name="pps", bufs=3, space="PSUM"))
    ppy = ph2.enter_context(tc.tile_pool(name="ppy", bufs=2, space="PSUM"))
    pprs = ph2.enter_context(tc.tile_pool(name="pprs", bufs=1, space="PSUM"))
    ppo = ph2.enter_context(tc.tile_pool(name="ppo", bufs=2, space="PSUM"))

    shared = scratch["shared"]
    cmask_sb = shared["cmask_sb"]
    wo_c = shared["wo_c"]

    octr = [0]
    oq = deque()  # pending output-projection quads from completed blocks

    def emit_one_quad(item):
        """One (u, ob) output tile: 4 accumulating matmuls + copy + store."""
        y_t, b, qb, u, ob = item
        tt0 = b * S + qb * TB + u * 128
        p_o = ppo.tile([128, TB], F32, tag="po", name="p_o")
        for c in range(QH):
            nc.tensor.matmul(
                p_o,
                y_t[:, c, u * 128 : (u + 1) * 128],
                wo_c[c][:, ob * TB : (ob + 1) * TB],
                start=(c == 0),
                stop=(c == QH - 1),
                skip_group_check=True,
            )
        o_sb = op.tile([128, TB], BF16, tag="osb", name="o_sb")
        if octr[0] % 2 == 0:
            nc.vector.tensor_copy(o_sb, p_o)
        else:
            nc.scalar.copy(o_sb, p_o)
        octr[0] += 1
        nc.sync.dma_start(out[tt0 : tt0 + 128, ob * TB : (ob + 1) * TB], o_sb)

    def push_block_quads(st):
        for u in range(TB // 128):
            for ob in range(D // TB):
                oq.append((st["y_t"], st["b"], st["qb"], u, ob))

    def emit_chain(st):
        """Scores + exp + mask + DVE rowsum accum + PV for one head, with
        output-projection quads drizzled into the PE gaps."""
        qb = st["qb"]
        kt_b, vt_b = st["kt_b"], st["vt_b"]
        qblk = st["qblk"]
        nkt = (qb + 1) * (TB // 128)
        p_y = ppy.tile([128, TB], F32, tag="py", name="p_y")
        e_sum = esp.tile([128, TB], BF16, tag="esum", name="e_sum")
        for kt in range(nkt):
            dj = kt - qb * (TB // 128)
            # diagonal k-tiles only contribute to tq >= tk: narrow the
            # streamed width to the valid 128-multiple column range
            if dj <= 0:
                c0 = 0
                mrange = None
            else:
                c0 = 128 * dj
                mrange = (c0, min(c0 + 128, TB))
            if dj == 0:
                mrange = (0, 128)
            p_s = pps.tile([128, TB], F32, tag="ps", name="p_s")
            nc.tensor.matmul(
                p_s[:, c0:],
                kt_b[:, kt * 128 : (kt + 1) * 128],
                qblk[:, c0:],
                start=True,
                stop=True,
            )
            # fill the PE while ScalarE computes this tile's exp
            if oq:
                emit_one_quad(oq.popleft())
            # kt==0 writes exp straight into e_sum (doubles as the running
            # denominator accumulator); later tiles add in on the DVE
            if kt == 0:
                e_t = e_sum
            else:
                e_t = ep.tile([128, TB], BF16, tag="et", name="e_t")
            nc.scalar.activation(e_t[:, c0:], p_s[:, c0:], Exp, scale=SCALE)
            if mrange is not None:
                m0, m1 = mrange
                nc.vector.tensor_mul(
                    e_t[:, m0:m1],
                    e_t[:, m0:m1],
                    cmask_sb[:, dj * TB + m0 : dj * TB + m1],
                )
            if kt > 0:
                nc.vector.tensor_add(
                    e_sum[:, c0:], e_sum[:, c0:], e_t[:, c0:]
                )
            nc.tensor.matmul(
                p_y[:, c0:],
                vt_b[:, kt, :],
                e_t[:, c0:],
                start=(kt == 0),
                stop=(kt == nkt - 1),
                skip_group_check=True,
            )
        st["p_y"] = p_y
        st["e_sum"] = e_sum

    def emit_rowsum(st):
        """Full-ones stationary: reduces e_sum over partitions AND broadcasts
        the rowsum to all 128 partitions in one matmul; fast reciprocal."""
        p_rsb = pprs.tile([128, TB], F32, tag="prs", name="p_rsb")
        nc.tensor.matmul(
            p_rsb, ones_t, st["e_sum"], start=True, stop=True,
            skip_group_check=True,
        )
        bc_sb = sp2.tile([128, TB], F32, tag="bc", name="bc_sb")
        nc.vector.reciprocal_approx_fast(out=bc_sb, in_=p_rsb)
        st["bc"] = bc_sb

    def emit_norm(st):
        nc.vector.tensor_mul(st["y_t"][:, st["h"], :], st["p_y"], st["bc"])

    chains = []
    for b in range(B):
        for qb in range(S // TB):
            for h in range(QH):
                chains.append(dict(idx=len(chains), b=b, qb=qb, h=h))

    y_t = None
    for st in chains:
        idx = st["idx"]
        # kv(b=0) was prefetched during phase 1; kv(b=1) loads early here
        if idx == 4:
            nc.scalar.dma_start(shared["kt"][1], kTs[1])
            nc.scalar.dma_start(
                shared["vt"][1], vs[1].rearrange("(n p) d -> p n d", p=128)
            )
        st["kt_b"] = shared["kt"][st["b"]]
        st["vt_b"] = shared["vt"][st["b"]]
        st["qblk"] = shared["qt"][st["b"]][
            :, st["h"], st["qb"] * TB : (st["qb"] + 1) * TB
        ]
        # stage C: normalize chain idx-2; a completed block queues its quads
        if idx >= 2:
            done = chains[idx - 2]
            emit_norm(done)
            if done["h"] == QH - 1:
                push_block_quads(done)
        # stage A: this chain's attention
        if st["h"] == 0:
            y_t = yp.tile([128, QH, TB], BF16, tag="yt", name="y_t")
        st["y_t"] = y_t
        emit_chain(st)
        # stage B: rowsum + reciprocal for chain idx-1
        if idx >= 1:
            emit_rowsum(chains[idx - 1])
    # drain the pipeline
    emit_rowsum(chains[-1])
    emit_norm(chains[-2])
    emit_norm(chains[-1])
    push_block_quads(chains[-1])
    while oq:
        emit_one_quad(oq.popleft())


def _build_nc(phases=(1, 2, 3)):
    import concourse.bass as bass  # noqa: F401
    import concourse.tile as tile
    from concourse import bacc, mybir

    F32 = mybir.dt.float32
    BF16 = mybir.dt.bfloat16

    nc = bacc.Bacc("TRN2", target_bir_lowering=False, debug=False, num_devices=NCORES)

    tens = {}
    tens["xT"] = nc.dram_tensor("xT", [D, T], BF16, kind="ExternalInput").ap()
    tens["wqkvT"] = nc.dram_tensor(
        "wqkvT", [D, EQ + 2 * HD], BF16, kind="ExternalInput"
    ).ap()
    tens["woT"] = nc.dram_tensor("woT", [EQ, D], BF16, kind="ExternalInput").ap()
    tens["cosT"] = nc.dram_tensor("cosT", [HD, S], F32, kind="ExternalInput").ap()
    tens["sinT"] = nc.dram_tensor("sinT", [HD, S], F32, kind="ExternalInput").ap()
    tens["cmask"] = nc.dram_tensor(
        "cmask", [128, 4 * TB], BF16, kind="ExternalInput"
    ).ap()
    tens["identr"] = nc.dram_tensor(
        "identr", [128, 128], BF16, kind="ExternalInput"
    ).ap()
    tens["ones"] = nc.dram_tensor("ones", [128, 128], BF16, kind="ExternalInput").ap()
    tens["out"] = nc.dram_tensor("out", [T, D], BF16, kind="ExternalOutput").ap()

    with tile.TileContext(nc) as tc, ExitStack() as top:
        dram = top.enter_context(tc.tile_pool(name="dram", bufs=1, space="DRAM"))
        scratch = {
            "kTs": [
                dram.tile([HD, S], BF16, name="kTs0"),
                dram.tile([HD, S], BF16, name="kTs1"),
            ],
            "vs": [
                dram.tile([S, HD], BF16, name="vs0"),
                dram.tile([S, HD], BF16, name="vs1"),
            ],
        }

        consts = top.enter_context(tc.tile_pool(name="consts", bufs=1))
        ones_t = consts.tile([128, 128], BF16)
        nc.sync.dma_start(ones_t, tens["ones"])
        tens["ones_t"] = ones_t
        identr_t = consts.tile([128, 128], BF16)
        nc.sync.dma_start(identr_t, tens["identr"])
        tens["identr_t"] = identr_t

        # persistent phase-2 working set: q lives in SBUF end to end; kv/wo
        # are prefetched from inside phase 1
        kvw = top.enter_context(tc.tile_pool(name="kvw", bufs=1))
        scratch["shared"] = {
            "qt": [
                kvw.tile([128, QH, S], BF16, tag=f"qt{b}", name=f"qt_b{b}")
                for b in range(B)
            ],
            "kt": [
                kvw.tile([128, S], BF16, tag=f"kt{b}", name=f"kt_b{b}")
                for b in range(B)
            ],
            "vt": [
                kvw.tile([128, NKT, 128], BF16, tag=f"vt{b}", name=f"vt_b{b}")
                for b in range(B)
            ],
            "wo_c": [
                kvw.tile([128, D], BF16, tag=f"woc{c}", name=f"wo_c{c}")
                for c in range(QH)
            ],
            "cmask_sb": kvw.tile([128, 4 * TB], BF16, tag="cm", name="cmask_sb"),
        }
        # pure input with no producers — load before phase 1 starts so the
        # first diagonal mask multiply never waits at the phase boundary
        nc.sync.dma_start(scratch["shared"]["cmask_sb"], tens["cmask"])

        if 1 in phases:
            with ExitStack() as ph1:
                _emit_phase1(nc, tc, ph1, mybir, tens, scratch)

        if 2 in phases:
            with ExitStack() as ph2:
                _emit_phase23(nc, tc, ph2, mybir, tens, scratch)

    nc.compile()
    return nc


def _get_nc():
    if "nc" not in _NC_CACHE:
        _NC_CACHE["nc"] = _build_nc()
    return _NC_CACHE["nc"]


def _host_prep(x, freqs_cos, freqs_sin, wq, wk, wv, wo):
    """Build per-core input maps (numpy only)."""
    import ml_dtypes

    BF = ml_dtypes.bfloat16
    x2d = np.ascontiguousarray(x.reshape(T, D).T).astype(BF)  # [D, T]

    # de-interleave permutation within each head: [r0..r63, i0..i63]
    perm = np.concatenate([np.arange(0, HD, 2), np.arange(1, HD, 2)])

    wq_h = wq.reshape(NH, HD, D)[:, perm, :].reshape(NH * HD, D)
    wk_h = wk.reshape(NKV, HD, D)[:, perm, :].reshape(NKV * HD, D)

    cos_de = np.empty((HD, S), np.float32)
    sin_de = np.empty((HD, S), np.float32)
    ft = freqs_cos.T  # [HD/2, S]
    st = freqs_sin.T
    cos_de[0:64] = ft
    cos_de[64:128] = ft
    sin_de[0:64] = -st
    sin_de[64:128] = st

    cmask = np.zeros((128, 4 * TB), np.float32)
    p = np.arange(128)[:, None]
    f = np.arange(TB)[None, :]
    for j in range(4):
        cmask[:, j * TB : (j + 1) * TB] = (p <= f - 128 * j).astype(np.float32)

    ones = np.ones((128, 128), np.float32)
    identr = np.eye(128, dtype=np.float32)

    in_maps = []
    for i in range(NCORES):
        qs = slice(i * EQ, (i + 1) * EQ)
        ks = slice(i * HD, (i + 1) * HD)
        wqkv = np.concatenate([wq_h[qs], wk_h[ks], wv[ks]], axis=0)  # [768, D]
        in_maps.append(
            dict(
                xT=x2d,
                wqkvT=np.ascontiguousarray(wqkv.T).astype(BF),
                woT=np.ascontiguousarray(wo[:, qs].T).astype(BF),
                cosT=cos_de,
                sinT=sin_de,
                cmask=cmask.astype(BF),
                ones=ones.astype(BF),
                identr=identr.astype(BF),
            )
        )
    return in_maps


def _numpy_fallback(x, freqs_cos, freqs_sin, wq, wk, wv, wo, k_cache, v_cache,
                    input_pos, mask):
    """Exact port of the reference for unexpected inputs. Slow but correct."""
    NREP = NH // NKV
    q = (x.reshape(T, D) @ wq.T).reshape(B, S, NH, HD)
    k = (x.reshape(T, D) @ wk.T).reshape(B, S, NKV, HD)
    v = (x.reshape(T, D) @ wv.T).reshape(B, S, NKV, HD)

    def rot(t):
        tr = t.reshape(*t.shape[:-1], HD // 2, 2)
        t_r, t_i = tr[..., 0], tr[..., 1]
        c = freqs_cos[None, :, None, :]
        s = freqs_sin[None, :, None, :]
        o_r = t_r * c - t_i * s
        o_i = t_r * s + t_i * c
        return np.stack([o_r, o_i], axis=-1).reshape(t.shape)

    q = rot(q).transpose(0, 2, 1, 3)
    k = rot(k).transpose(0, 2, 1, 3)
    v = v.transpose(0, 2, 1, 3)
    k_full = np.array(k_cache)
    v_full = np.array(v_cache)
    k_full[:, :, input_pos] = k
    v_full[:, :, input_pos] = v
    k_rep = np.repeat(k_full, NREP, axis=1)
    v_rep = np.repeat(v_full, NREP, axis=1)
    am = mask[input_pos][None, None]
    scores = np.einsum("bhqd,bhkd->bhqk", q, k_rep, optimize=True) * SCALE
    scores = np.where(am, scores, -np.inf)
    scores -= scores.max(axis=-1, keepdims=True)
    e = np.exp(scores)
    probs = e / e.sum(axis=-1, keepdims=True)
    y = np.einsum("bhqk,bhkd->bhqd", probs, v_rep, optimize=True)
    y = y.transpose(0, 2, 1, 3).reshape(B, S, NH * HD)
    return (y @ wo.T).astype(np.float32)


def kernel(**inputs):
    x = np.asarray(inputs["x"], np.float32)
    freqs_cos = np.asarray(inputs["freqs_cos"], np.float32)
    freqs_sin = np.asarray(inputs["freqs_sin"], np.float32)
    wq = np.asarray(inputs["wq"], np.float32)
    wk = np.asarray(inputs["wk"], np.float32)
    wv = np.asarray(inputs["wv"], np.float32)
    wo = np.asarray(inputs["wo"], np.float32)
    input_pos = np.asarray(inputs["input_pos"])
    mask = np.asarray(inputs["mask"])

    std = (
        np.array_equal(input_pos, np.arange(S, dtype=input_pos.dtype))
        and bool((mask == np.tril(np.ones((S, S), bool))).all())
    )
    if not std:
        return _numpy_fallback(
            x, freqs_cos, freqs_sin, wq, wk, wv, wo,
            inputs["k_cache"], inputs["v_cache"], input_pos, mask,
        )

    from concourse.bass_utils import run_bass_kernel_spmd

    nc = _get_nc()
    in_maps = _host_prep(x, freqs_cos, freqs_sin, wq, wk, wv, wo)
    res = run_bass_kernel_spmd(nc, in_maps, core_ids=list(range(NCORES)))
    acc = res.results[0]["out"].astype(np.float32)
    for r in res.results[1:]:
        acc = acc + r["out"].astype(np.float32)
    return acc.reshape(B, S, D).astype(np.float32)
